# revision 47
# baseline (speedup 1.0000x reference)
"""Trainium2 Bass kernel for nn_AttentionFusion (8-core data-parallel over B).

Reference computation per batch b:
    p_proj = X @ W_p + b_p                      # (N, C)
    c_proj = CF @ W_c + b_c                     # (NC, C)
    S      = p_proj @ c_proj.T                  # (N, NC)
    W      = softmax(S, axis=-1)
    out    = X + W @ CF                         # (N, C)

Algebraic refactor (exact in real arithmetic):
    S = X @ M + 1·t  with  M = W_p @ c_proj.T (C×NC),  t = b_p @ c_proj.T (NC)
so the (N,C)x(C,C) projection matmul disappears.  M and t depend only on
the (tiny) weights, so they are folded on the host in fp64 — the same
weight-folding the algebraic rewrite already does, just ahead of time —
which removes the whole on-device setup matmul chain from the critical
path.

X is round-to-nearest cast to bf16 on the host and bound to a bf16 DRAM
parameter (halves X's HBM traffic; bf16 PE transposes are single-pass,
fp32 ones are double-pass).  The output is written bf16 and upcast on
the host (halves output traffic).  Total HBM traffic: 16 MiB/core.

Scores are computed TRANSPOSED (S^T; M's 64 columns are duplicated so
S^T lands twice, partitions 0-63 / 64-127) in bf16 at 1 cyc/row, exp's
+t bias is per-partition, and exp(S^T) is written BF16 so the
weighted-sum matmul runs the fast bf16 LDWEIGHTS/MM path (the fp32/f32r
stationary path costs ~2x per matmul).  The weighted rhs is host-built
bf16: rows 0-63 = [bf16(CF) | 1 | 1], rows 64-127 =
[bf16(CF - bf16(CF)) | 0 | 0] — the k-duplicated expT contracts hi+lo,
so CF keeps ~16 mantissa bits and the same matmul also emits the
softmax normalizer.

Per-core engine split per 1024-row supertile: PE does 16 back-to-back
bf16 transposes, 4 scores and 8 weighted matmuls (dense bursts keep the
PE activity monitor at full clock); DVE does the single 2x-rate bf16
X^T copy, pair reciprocals, and fused (ws*recip)+x for chunks 0-3; ACT
does two 512-wide exps and normalize muls for chunks 4-7; GPSIMD adds
the residual for chunks 4-7 in one batched op.

DMA routing: ALL x loads ride the Sync HWDGE ring (FIFO, so x(0) lands
without contention and compute starts ~4us earlier); outputs ride the
GPSIMD SWDGE queue until input issue is done, then switch to Sync; the
final supertile drains as four 128KiB per-pair DMAs on Sync with
all-DVE postprocess.

Note on scheduling: the Tile schedule is extremely sensitive —
seemingly-neutral restructurings (per-pair DVE/ACT interleave, 1-op
exp, tighter pools, half-supertile PSUM granularity) measured 3-10%
WORSE by decoupling the front-end from the back-end (HAM re-throttles
the PE) or adding cross-engine sem hops.  Measure 3+ runs: run-to-run
noise is +-2.5us and the first run after a rebuild is often an outlier.

Sharding: B=8 batches -> one batch per NeuronCore, weights replicated.
"""

import numpy as np

B, N, NC, C = 8, 16384, 64, 256
P = 128  # SBUF partitions
SUPER_ROWS = 1024  # rows per DMA supertile (row = s*1024 + p*8 + j)
JCHUNK = SUPER_ROWS // P  # 8 row-chunks per supertile
HALF = 4  # chunks per scores tile (4*128 = 512 rows)
NSUPER = N // SUPER_ROWS

_CACHE = {}


def _split_multiwait_ctrl(nc, mybir):
    """This toolchain's walrus accepts only ONE sync wait per instruction,
    but Tile's scheduler attaches one wait per depended-on proc.  Keep the
    last wait on the instruction and hoist the excess onto single-wait NoOps
    inserted immediately before it on the same engine (same sequencer order,
    identical blocking semantics)."""
    for f in nc.m.functions:
        for bb in f.blocks:
            insts = bb.instructions
            new_list = []
            changed = False
            for inst in insts:
                si = inst.sync_info
                if si is not None and si.on_wait and len(si.on_wait) > 1:
                    waits = list(si.on_wait)
                    for w in waits[:-1]:
                        nop = mybir.InstNoOp(
                            name=nc.get_next_instruction_name(),
                            engine=inst.engine,
                            sync_info=mybir.SyncInfo(on_wait=[w], on_update=[]),
                            bass_nofuse=True,
                        )
                        nc.register_instruction(nop, overwrite=True)
                        new_list.append(nop)
                        changed = True
                    inst.sync_info = mybir.SyncInfo(
                        on_wait=[waits[-1]], on_update=list(si.on_update or [])
                    )
                new_list.append(inst)
            if changed:
                bb.instructions[:] = new_list
    return nc


def _build():
    from contextlib import ExitStack

    import concourse.bass as bass
    import concourse.mybir as mybir
    import concourse.tile as tile
    from concourse.masks import make_identity

    f32 = mybir.dt.float32
    f32r = mybir.dt.float32r
    bf16 = mybir.dt.bfloat16
    Exp = mybir.ActivationFunctionType.Exp

    nc = bass.Bass("TRN2", target_bir_lowering=False, debug=False)
    x = nc.declare_dram_parameter("x", [N, C], bf16, isOutput=False)
    cfs = nc.declare_dram_parameter("cfs", [P, C + 2], bf16, isOutput=False)
    mcd = nc.declare_dram_parameter("mcd", [C, 2 * NC], bf16, isOutput=False)
    td = nc.declare_dram_parameter("td", [P, 1], f32, isOutput=False)
    out = nc.declare_dram_parameter("out", [N, C], bf16, isOutput=True)

    KC = C // P  # 2 contraction chunks of 128 over the C dim
    RW = HALF * P  # 512 rows per scores tile

    with tile.TileContext(nc) as tc:
        with (
            tc.tile_pool(name="const", bufs=1) as const,
            tc.tile_pool(name="xin", bufs=10) as xin,
            tc.tile_pool(name="oout", bufs=8) as oout,
            tc.tile_pool(name="work", bufs=6) as work,
        ):
            x_view = x.rearrange("(s p j) c -> s p j c", p=P, j=JCHUNK)
            o_view = out.rearrange("(s p j) c -> s p j c", p=P, j=JCHUNK)

            x_tiles = [None] * NSUPER
            NPRE = 6

            def load_x(s, engine=None):
                x_tiles[s] = xin.tile(
                    [P, JCHUNK, C], bf16, tag="x_tile", name=f"x_tile{s}"
                )
                (engine or nc.sync).dma_start(x_tiles[s], x_view[s])

            # ---------------- setup: constants (host-folded M, t, CF) -------
            # All x loads ride the Sync HWDGE ring: the ring is FIFO, so x(0)
            # issued first lands with no contention from the prefetches that
            # queue behind it.  Tiny constants go on the scalar ring in
            # parallel.
            load_x(0)
            mc_sb = const.tile([P, KC, 2 * NC], bf16)
            nc.scalar.dma_start(mc_sb, mcd.rearrange("(k p) n -> p k n", p=P))
            tT = const.tile([P, 1], f32)
            nc.scalar.dma_start(tT, td.ap())
            # cfstack [128, C+2] bf16 (host-built): rows 0-63 = [bf16(CF)|1|1],
            # rows 64-127 = [bf16(CF - bf16(CF)) | 0 | 0]; the k-duplicated
            # expT contracts hi+lo so CF keeps ~16 mantissa bits.
            cfstack = const.tile([P, C + 2], bf16)
            nc.scalar.dma_start(cfstack, cfs.ap())
            for s in range(1, NPRE):
                load_x(s)

            ident = const.tile([P, P], f32)
            make_identity(nc, ident)
            identb = const.tile([P, P], bf16)
            nc.vector.tensor_copy(identb, ident)
            ident2 = const.tile([P, P], f32)
            nc.vector.tensor_copy(ident2, ident)

            setup_stack = ExitStack()
            setup_ps = setup_stack.enter_context(
                tc.tile_pool(name="setup_ps", bufs=1, space="PSUM")
            )
            # Preload the exp table while the constant DMAs land so the
            # first real exp doesn't eat the ~1.3us ACT_TABLE_LOAD.
            dummy = const.tile([P, 1], f32)
            nc.scalar.activation(dummy, ident[:, :1], Exp)
            # Warm the PE clock gate (fp32 transposes; distinct
            # source/identity tiles — aliased operands hang the HW) so the
            # first supertiles run closer to full clock.  Sized to roughly
            # cover the x(0) DMA latency — real work takes over from there.
            warm_ps = setup_ps.tile([P, P], f32, tag="warm")
            for _ in range(10):
                nc.tensor.transpose(warm_ps, ident, ident2)
            setup_stack.close()

            # ---------------- main loop --------------------------------------
            ps_stack = ExitStack()
            ps_xt = ps_stack.enter_context(
                tc.tile_pool(name="ps_xt", bufs=1, space="PSUM")
            )
            ps_sc = ps_stack.enter_context(
                tc.tile_pool(name="ps_sc", bufs=1, space="PSUM")
            )
            ps_ws = ps_stack.enter_context(
                tc.tile_pool(name="ps_ws", bufs=2, space="PSUM")
            )

            exp_tiles = [None] * NSUPER
            xt_sbs = [None] * NSUPER

            def front_a(s):
                """Transposes + X^T copy for supertile s."""
                x_tile = x_tiles[s]
                # X^T for the whole 1024-row supertile: 16 back-to-back PE
                # transposes into one 2-bank bf16 PSUM tile, then a single
                # 2x-packed DVE copy
                # (free = jj*128 + p <-> row s*1024 + p*8 + jj)
                xt_ps = ps_xt.tile([P, KC, 2 * RW], bf16, tag="xt")
                for k in range(KC):
                    for jj in range(JCHUNK):
                        nc.tensor.transpose(
                            xt_ps[:, k, bass.ts(jj, P)],
                            x_tile[:, jj, bass.ts(k, P)],
                            identb,
                        )
                xt_sb = work.tile([P, KC, 2 * RW], bf16, tag="xt_sb")
                nc.vector.tensor_copy(xt_sb, xt_ps)
                xt_sbs[s] = xt_sb

            def front_b(s):
                """Scores + exp for supertile s."""
                xt_sb = xt_sbs[s]
                # S^T[k, r] = sum_c M[c,k] X[r,c]  (k duplicated 2x),
                # two 512-row groups
                sc_ps = ps_sc.tile([P, 2, RW], f32, tag="sc")
                for g in range(2):
                    for k in range(KC):
                        nc.tensor.matmul(
                            sc_ps[:, g, :],
                            mc_sb[:, k, :],
                            xt_sb[:, k, bass.ts(g, RW)],
                            start=(k == 0),
                            stop=(k == KC - 1),
                        )

                # expT = exp(S^T + t), one ACT op per 512-row group.
                # bf16 output: the weighted matmul then runs the fast bf16
                # LDWEIGHTS/MM path instead of the 2x-slower fp32 one.
                expT = work.tile([P, 2, RW], bf16, tag="expT")
                for g in range(2):
                    nc.scalar.activation(expT[:, g], sc_ps[:, g], Exp, bias=tT)
                exp_tiles[s] = expT

            def back(s, mid=None):
                """Weighted matmuls + normalize + residual + store for s."""
                x_tile = x_tiles[s]
                expT = exp_tiles[s]
                o_tile = oout.tile([P, JCHUNK, C], bf16, tag="o_tile")

                # weighted[r, c] = sum_k expT[k,r] [CF|1][k,c]; four 2-bank
                # PSUM pair-tiles per supertile, double-buffered
                for pair in range(4):
                    if pair == 2 and mid is not None:
                        # hoist the NEXT supertile's dependency-free
                        # transposes+copy here: the 16 transposes fill the
                        # PE bubble while pair 2 waits on pair 0's
                        # postprocess, instead of queueing behind all 8
                        # weighted matmuls in the PE FIFO
                        mid()
                    ws = ps_ws.tile([P, 2, 512], f32, tag="ws")
                    for jj2 in range(2):
                        jj = pair * 2 + jj2
                        nc.tensor.matmul(
                            ws[:, jj2, : C + 2],
                            expT[:, jj // HALF, bass.ts(jj % HALF, P)],
                            cfstack,
                            start=True,
                            stop=True,
                        )
                    recip = work.tile([P, 2], f32, tag=f"recip{pair}")
                    nc.vector.reciprocal(recip, ws[:, :, C])
                    # pairs 0-1 drain on DVE (fused stt), pairs 2-3 on ACT
                    # (mul, residual added by GPSIMD): each engine works
                    # through its own pairs with no cross-engine handoff
                    # inside a pair.  On the final supertile every chunk is
                    # a DVE stt so the drain skips the ACT->GPSIMD chain.
                    for jj2 in range(2):
                        jj = pair * 2 + jj2
                        if jj < HALF or s == NSUPER - 1:
                            # fused (ws*recip)+x on DVE
                            nc.vector.scalar_tensor_tensor(
                                o_tile[:, jj, :],
                                ws[:, jj2, :C],
                                recip[:, jj2 : jj2 + 1],
                                x_tile[:, jj, :],
                                op0=mybir.AluOpType.mult,
                                op1=mybir.AluOpType.add,
                            )
                        else:
                            nc.scalar.mul(
                                o_tile[:, jj, :],
                                ws[:, jj2, :C],
                                recip[:, jj2 : jj2 + 1],
                            )
                    if s == NSUPER - 1:
                        # drain mode: ship each 128KiB pair-output on the
                        # now-idle Sync HWDGE ring as soon as its two DVE
                        # stts land, so the final DMA is tiny
                        nc.sync.dma_start(
                            o_view[s][:, 2 * pair : 2 * pair + 2],
                            o_tile[:, 2 * pair : 2 * pair + 2],
                        )
                    elif pair == 3:
                        # batched residual for the jj=4..7 ACT-mul chunks
                        nc.gpsimd.tensor_add(
                            o_tile[:, HALF:],
                            o_tile[:, HALF:],
                            x_tile[:, HALF:],
                        )
                        # outputs ride the GPSIMD SWDGE queue while the
                        # Sync ring carries input loads; once input issue
                        # is done the last few outputs switch to the
                        # lower-latency Sync ring so the SWDGE queue has
                        # drained before the final supertile lands.
                        if s >= NSUPER - 3:
                            nc.sync.dma_start(o_view[s], o_tile)
                        else:
                            nc.gpsimd.dma_start(o_view[s], o_tile)

            # Emission: scores+exp of s, then the pair loop of s with the
            # NEXT supertile's transposes+copy hoisted between pairs 1 and
            # 2 (see back()).  (Full lag-1 pipelining — the whole front of
            # s+1 before back(s) — measured ~4us WORSE; this partial hoist
            # moves only the dependency-free part.)
            front_a(0)
            for s in range(NSUPER):
                # hoisted prefetch: the Sync ring carries only inputs,
                # and this DMA's buffer-reuse wait is satisfied at issue
                if s + NPRE < NSUPER and x_tiles[s + NPRE] is None:
                    load_x(s + NPRE)
                front_b(s)
                back(
                    s,
                    mid=(lambda t=s + 1: front_a(t))
                    if s + 1 < NSUPER
                    else None,
                )

            ps_stack.close()

    return _split_multiwait_ctrl(nc, mybir)


def _get_nc():
    if "nc" not in _CACHE:
        _CACHE["nc"] = _build()
    return _CACHE["nc"]


def run(inputs, trace=False):
    import ml_dtypes

    from concourse.bass_utils import run_bass_kernel_spmd

    nc = _get_nc()
    pf = np.ascontiguousarray(
        np.asarray(inputs["point_features"], dtype=np.float32)
    ).astype(ml_dtypes.bfloat16)
    cfeat = np.ascontiguousarray(
        np.asarray(inputs["centroid_features"], dtype=np.float32)
    )
    wp = np.asarray(inputs["W_p"], dtype=np.float64)
    bp = np.asarray(inputs["b_p"], dtype=np.float64)
    wc = np.asarray(inputs["W_c"], dtype=np.float64)
    bc = np.asarray(inputs["b_c"], dtype=np.float64)

    # Host-fold the weight-only constants (fp64): M = W_p @ c_proj.T,
    # t = b_p @ c_proj.T, duplicated along k so S^T lands twice.
    # cfs [128, C+2] bf16: rows 0-63 = [bf16(CF) | 1 | 1], rows 64-127 =
    # [bf16(CF - bf16(CF)) | 0 | 0]; the duplicated expT contracts hi+lo
    # so CF keeps ~16 mantissa bits through the bf16 weighted matmul.
    in_maps = []
    for b in range(B):
        cproj = cfeat[b].astype(np.float64) @ wc + bc  # (NC, C)
        m = (wp @ cproj.T).astype(ml_dtypes.bfloat16)  # (C, NC)
        t = (bp @ cproj.T).astype(np.float32)  # (NC,)
        mcd = np.ascontiguousarray(np.concatenate([m, m], axis=1))
        td = np.concatenate([t, t]).reshape(P, 1)
        cf64 = cfeat[b].astype(np.float64)  # (NC, C)
        cf_hi = cf64.astype(ml_dtypes.bfloat16)
        cf_lo = (cf64 - cf_hi.astype(np.float64)).astype(ml_dtypes.bfloat16)
        cfs = np.zeros((P, C + 2), dtype=ml_dtypes.bfloat16)
        cfs[:NC, :C] = cf_hi
        cfs[NC:, :C] = cf_lo
        cfs[:NC, C:] = ml_dtypes.bfloat16(1.0)
        in_maps.append(
            {"x": pf[b], "cfs": cfs, "mcd": mcd, "td": td}
        )
    res = run_bass_kernel_spmd(nc, in_maps, core_ids=list(range(B)), trace=trace)
    out = np.stack(
        [np.asarray(res.results[b]["out"]).astype(np.float32) for b in range(B)],
        axis=0,
    )
    return out, res


def kernel(**inputs) -> np.ndarray:
    out, _ = run(inputs, trace=False)
    return out



# revision 49
# speedup vs baseline: 1.0034x; 1.0034x over previous
"""Trainium2 Bass kernel for nn_AttentionFusion (8-core data-parallel over B).

Reference computation per batch b:
    p_proj = X @ W_p + b_p                      # (N, C)
    c_proj = CF @ W_c + b_c                     # (NC, C)
    S      = p_proj @ c_proj.T                  # (N, NC)
    W      = softmax(S, axis=-1)
    out    = X + W @ CF                         # (N, C)

Algebraic refactor (exact in real arithmetic):
    S = X @ M + 1·t  with  M = W_p @ c_proj.T (C×NC),  t = b_p @ c_proj.T (NC)
so the (N,C)x(C,C) projection matmul disappears.  M and t depend only on
the (tiny) weights, so they are folded on the host in fp64 — the same
weight-folding the algebraic rewrite already does, just ahead of time —
which removes the whole on-device setup matmul chain from the critical
path.

X is round-to-nearest cast to bf16 on the host and bound to a bf16 DRAM
parameter (halves X's HBM traffic; bf16 PE transposes are single-pass,
fp32 ones are double-pass).  The output is written bf16 and upcast on
the host (halves output traffic).  Total HBM traffic: 16 MiB/core.

Scores are computed TRANSPOSED (S^T; M's 64 columns are duplicated so
S^T lands twice, partitions 0-63 / 64-127) in bf16 at 1 cyc/row, exp's
+t bias is per-partition, and exp(S^T) is written BF16 so the
weighted-sum matmul runs the fast bf16 LDWEIGHTS/MM path (the fp32/f32r
stationary path costs ~2x per matmul).  The weighted rhs is host-built
bf16: rows 0-63 = [bf16(CF) | 1 | 1], rows 64-127 =
[bf16(CF - bf16(CF)) | 0 | 0] — the k-duplicated expT contracts hi+lo,
so CF keeps ~16 mantissa bits and the same matmul also emits the
softmax normalizer.

Per-core engine split per 1024-row supertile: PE does 16 back-to-back
bf16 transposes, 4 scores and 8 weighted matmuls (dense bursts keep the
PE activity monitor at full clock); DVE does the single 2x-rate bf16
X^T copy, pair reciprocals, and fused (ws*recip)+x for chunks 0-3; ACT
does two 512-wide exps and normalize muls for chunks 4-7; GPSIMD adds
the residual for chunks 4-7 in one batched op.

DMA routing: ALL x loads ride the Sync HWDGE ring (FIFO, so x(0) lands
without contention and compute starts ~4us earlier); outputs ride the
GPSIMD SWDGE queue until input issue is done, then switch to Sync; the
final supertile drains as four 128KiB per-pair DMAs on Sync with
all-DVE postprocess.

Note on scheduling: the Tile schedule is extremely sensitive —
seemingly-neutral restructurings (per-pair DVE/ACT interleave, 1-op
exp, tighter pools, half-supertile PSUM granularity) measured 3-10%
WORSE by decoupling the front-end from the back-end (HAM re-throttles
the PE) or adding cross-engine sem hops.  Measure 3+ runs: run-to-run
noise is +-2.5us and the first run after a rebuild is often an outlier.

Sharding: B=8 batches -> one batch per NeuronCore, weights replicated.
"""

import numpy as np

B, N, NC, C = 8, 16384, 64, 256
P = 128  # SBUF partitions
SUPER_ROWS = 1024  # rows per DMA supertile (row = s*1024 + p*8 + j)
JCHUNK = SUPER_ROWS // P  # 8 row-chunks per supertile
HALF = 4  # chunks per scores tile (4*128 = 512 rows)
NSUPER = N // SUPER_ROWS

_CACHE = {}


def _split_multiwait_ctrl(nc, mybir):
    """This toolchain's walrus accepts only ONE sync wait per instruction,
    but Tile's scheduler attaches one wait per depended-on proc.  Keep the
    last wait on the instruction and hoist the excess onto single-wait NoOps
    inserted immediately before it on the same engine (same sequencer order,
    identical blocking semantics)."""
    for f in nc.m.functions:
        for bb in f.blocks:
            insts = bb.instructions
            new_list = []
            changed = False
            for inst in insts:
                si = inst.sync_info
                if si is not None and si.on_wait and len(si.on_wait) > 1:
                    waits = list(si.on_wait)
                    for w in waits[:-1]:
                        nop = mybir.InstNoOp(
                            name=nc.get_next_instruction_name(),
                            engine=inst.engine,
                            sync_info=mybir.SyncInfo(on_wait=[w], on_update=[]),
                            bass_nofuse=True,
                        )
                        nc.register_instruction(nop, overwrite=True)
                        new_list.append(nop)
                        changed = True
                    inst.sync_info = mybir.SyncInfo(
                        on_wait=[waits[-1]], on_update=list(si.on_update or [])
                    )
                new_list.append(inst)
            if changed:
                bb.instructions[:] = new_list
    return nc


def _build():
    from contextlib import ExitStack

    import concourse.bass as bass
    import concourse.mybir as mybir
    import concourse.tile as tile
    from concourse.masks import make_identity

    f32 = mybir.dt.float32
    f32r = mybir.dt.float32r
    bf16 = mybir.dt.bfloat16
    Exp = mybir.ActivationFunctionType.Exp

    nc = bass.Bass("TRN2", target_bir_lowering=False, debug=False)
    x = nc.declare_dram_parameter("x", [N, C], bf16, isOutput=False)
    cfs = nc.declare_dram_parameter("cfs", [P, C + 2], bf16, isOutput=False)
    mcd = nc.declare_dram_parameter("mcd", [C, 2 * NC], bf16, isOutput=False)
    td = nc.declare_dram_parameter("td", [P, 1], f32, isOutput=False)
    out = nc.declare_dram_parameter("out", [N, C], bf16, isOutput=True)

    KC = C // P  # 2 contraction chunks of 128 over the C dim
    RW = HALF * P  # 512 rows per scores tile

    with tile.TileContext(nc) as tc:
        with (
            tc.tile_pool(name="const", bufs=1) as const,
            tc.tile_pool(name="xin", bufs=10) as xin,
            tc.tile_pool(name="oout", bufs=8) as oout,
            tc.tile_pool(name="work", bufs=6) as work,
        ):
            x_view = x.rearrange("(s p j) c -> s p j c", p=P, j=JCHUNK)
            o_view = out.rearrange("(s p j) c -> s p j c", p=P, j=JCHUNK)

            x_tiles = [None] * NSUPER
            NPRE = 6

            def load_x(s, engine=None):
                x_tiles[s] = xin.tile(
                    [P, JCHUNK, C], bf16, tag="x_tile", name=f"x_tile{s}"
                )
                (engine or nc.sync).dma_start(x_tiles[s], x_view[s])

            # ---------------- setup: constants (host-folded M, t, CF) -------
            # All x loads ride the Sync HWDGE ring: the ring is FIFO, so x(0)
            # issued first lands with no contention from the prefetches that
            # queue behind it.  Tiny constants go on the scalar ring in
            # parallel.
            load_x(0)
            mc_sb = const.tile([P, KC, 2 * NC], bf16)
            nc.scalar.dma_start(mc_sb, mcd.rearrange("(k p) n -> p k n", p=P))
            tT = const.tile([P, 1], f32)
            nc.scalar.dma_start(tT, td.ap())
            # cfstack [128, C+2] bf16 (host-built): rows 0-63 = [bf16(CF)|1|1],
            # rows 64-127 = [bf16(CF - bf16(CF)) | 0 | 0]; the k-duplicated
            # expT contracts hi+lo so CF keeps ~16 mantissa bits.
            cfstack = const.tile([P, C + 2], bf16)
            nc.scalar.dma_start(cfstack, cfs.ap())
            for s in range(1, NPRE):
                load_x(s)

            ident = const.tile([P, P], f32)
            make_identity(nc, ident)
            identb = const.tile([P, P], bf16)
            nc.vector.tensor_copy(identb, ident)
            ident2 = const.tile([P, P], f32)
            nc.vector.tensor_copy(ident2, ident)

            setup_stack = ExitStack()
            setup_ps = setup_stack.enter_context(
                tc.tile_pool(name="setup_ps", bufs=1, space="PSUM")
            )
            # Preload the exp table while the constant DMAs land so the
            # first real exp doesn't eat the ~1.3us ACT_TABLE_LOAD.
            dummy = const.tile([P, 1], f32)
            nc.scalar.activation(dummy, ident[:, :1], Exp)
            # Warm the PE clock gate (fp32 transposes; distinct
            # source/identity tiles — aliased operands hang the HW) so the
            # first supertiles run closer to full clock.  Sized to roughly
            # cover the x(0) DMA latency — real work takes over from there.
            warm_ps = setup_ps.tile([P, P], f32, tag="warm")
            for _ in range(10):
                nc.tensor.transpose(warm_ps, ident, ident2)
            setup_stack.close()

            # ---------------- main loop --------------------------------------
            ps_stack = ExitStack()
            ps_xt = ps_stack.enter_context(
                tc.tile_pool(name="ps_xt", bufs=1, space="PSUM")
            )
            ps_sc = ps_stack.enter_context(
                tc.tile_pool(name="ps_sc", bufs=1, space="PSUM")
            )
            ps_ws = ps_stack.enter_context(
                tc.tile_pool(name="ps_ws", bufs=2, space="PSUM")
            )

            exp_tiles = [None] * NSUPER
            xt_pss = [None] * NSUPER
            xt_sbs = [None] * NSUPER

            def front_a(s):
                """16 PE transposes of X into PSUM for supertile s."""
                x_tile = x_tiles[s]
                # X^T for the whole 1024-row supertile: 16 back-to-back PE
                # transposes into one 2-bank bf16 PSUM tile
                # (free = jj*128 + p <-> row s*1024 + p*8 + jj)
                xt_ps = ps_xt.tile([P, KC, 2 * RW], bf16, tag="xt")
                for k in range(KC):
                    for jj in range(JCHUNK):
                        nc.tensor.transpose(
                            xt_ps[:, k, bass.ts(jj, P)],
                            x_tile[:, jj, bass.ts(k, P)],
                            identb,
                        )
                xt_pss[s] = xt_ps

            def front_c(s):
                """Single 2x-packed DVE PSUM->SBUF copy of X^T for s.

                Emitted at the END of back(s-1), after all of s-1's recips
                and stts: placing it mid-back put it between the stts in
                DVE's FIFO, which delayed the ws-buffer release that pair
                2's matmuls wait on and stalled the PE ~1.3us/supertile.
                Here only the next scores (latency-tolerant, one supertile
                ahead) wait on it.
                """
                xt_sb = work.tile([P, KC, 2 * RW], bf16, tag="xt_sb")
                nc.vector.tensor_copy(xt_sb, xt_pss[s])
                xt_sbs[s] = xt_sb

            def front_b(s):
                """Scores + exp for supertile s."""
                xt_sb = xt_sbs[s]
                # S^T[k, r] = sum_c M[c,k] X[r,c]  (k duplicated 2x),
                # two 512-row groups
                sc_ps = ps_sc.tile([P, 2, RW], f32, tag="sc")
                for g in range(2):
                    for k in range(KC):
                        nc.tensor.matmul(
                            sc_ps[:, g, :],
                            mc_sb[:, k, :],
                            xt_sb[:, k, bass.ts(g, RW)],
                            start=(k == 0),
                            stop=(k == KC - 1),
                        )

                # expT = exp(S^T + t), one ACT op per 512-row group.
                # bf16 output: the weighted matmul then runs the fast bf16
                # LDWEIGHTS/MM path instead of the 2x-slower fp32 one.
                expT = work.tile([P, 2, RW], bf16, tag="expT")
                for g in range(2):
                    nc.scalar.activation(expT[:, g], sc_ps[:, g], Exp, bias=tT)
                exp_tiles[s] = expT

            def back(s, mid=None):
                """Weighted matmuls + normalize + residual + store for s."""
                x_tile = x_tiles[s]
                expT = exp_tiles[s]
                o_tile = oout.tile([P, JCHUNK, C], bf16, tag="o_tile")

                # weighted[r, c] = sum_k expT[k,r] [CF|1][k,c]; four 2-bank
                # PSUM pair-tiles per supertile, double-buffered
                for pair in range(4):
                    if pair == 2 and mid is not None:
                        # hoist the NEXT supertile's dependency-free
                        # transposes+copy here: the 16 transposes fill the
                        # PE bubble while pair 2 waits on pair 0's
                        # postprocess, instead of queueing behind all 8
                        # weighted matmuls in the PE FIFO
                        mid()
                    ws = ps_ws.tile([P, 2, 512], f32, tag="ws")
                    for jj2 in range(2):
                        jj = pair * 2 + jj2
                        nc.tensor.matmul(
                            ws[:, jj2, : C + 2],
                            expT[:, jj // HALF, bass.ts(jj % HALF, P)],
                            cfstack,
                            start=True,
                            stop=True,
                        )
                    recip = work.tile([P, 2], f32, tag=f"recip{pair}")
                    nc.vector.reciprocal(recip, ws[:, :, C])
                    # pairs 0-1 drain on DVE (fused stt), pairs 2-3 on ACT
                    # (mul, residual added by GPSIMD): each engine works
                    # through its own pairs with no cross-engine handoff
                    # inside a pair.  On the final supertile every chunk is
                    # a DVE stt so the drain skips the ACT->GPSIMD chain.
                    for jj2 in range(2):
                        jj = pair * 2 + jj2
                        if jj < HALF or s == NSUPER - 1:
                            # fused (ws*recip)+x on DVE
                            nc.vector.scalar_tensor_tensor(
                                o_tile[:, jj, :],
                                ws[:, jj2, :C],
                                recip[:, jj2 : jj2 + 1],
                                x_tile[:, jj, :],
                                op0=mybir.AluOpType.mult,
                                op1=mybir.AluOpType.add,
                            )
                        else:
                            nc.scalar.mul(
                                o_tile[:, jj, :],
                                ws[:, jj2, :C],
                                recip[:, jj2 : jj2 + 1],
                            )
                    if s == NSUPER - 1:
                        # drain mode: ship each 128KiB pair-output on the
                        # now-idle Sync HWDGE ring as soon as its two DVE
                        # stts land, so the final DMA is tiny
                        nc.sync.dma_start(
                            o_view[s][:, 2 * pair : 2 * pair + 2],
                            o_tile[:, 2 * pair : 2 * pair + 2],
                        )
                    elif pair == 3:
                        # batched residual for the jj=4..7 ACT-mul chunks
                        nc.gpsimd.tensor_add(
                            o_tile[:, HALF:],
                            o_tile[:, HALF:],
                            x_tile[:, HALF:],
                        )
                        # outputs ride the GPSIMD SWDGE queue while the
                        # Sync ring carries input loads; once input issue
                        # is done the last few outputs switch to the
                        # lower-latency Sync ring so the SWDGE queue has
                        # drained before the final supertile lands.
                        if s >= NSUPER - 3:
                            nc.sync.dma_start(o_view[s], o_tile)
                        else:
                            nc.gpsimd.dma_start(o_view[s], o_tile)

            # Emission: scores+exp of s, then the pair loop of s with the
            # NEXT supertile's transposes hoisted between pairs 1 and 2
            # (see back()) and its X^T copy emitted after the pair loop.
            # (Full lag-1 pipelining — the whole front of s+1 before
            # back(s) — measured ~4us WORSE; this partial hoist moves only
            # the dependency-free parts into the measured bubbles.)
            front_a(0)
            front_c(0)
            for s in range(NSUPER):
                # hoisted prefetch: the Sync ring carries only inputs,
                # and this DMA's buffer-reuse wait is satisfied at issue
                if s + NPRE < NSUPER and x_tiles[s + NPRE] is None:
                    load_x(s + NPRE)
                front_b(s)
                back(
                    s,
                    mid=(lambda t=s + 1: front_a(t))
                    if s + 1 < NSUPER
                    else None,
                )
                if s + 1 < NSUPER:
                    front_c(s + 1)

            ps_stack.close()

    return _split_multiwait_ctrl(nc, mybir)


def _get_nc():
    if "nc" not in _CACHE:
        _CACHE["nc"] = _build()
    return _CACHE["nc"]


def run(inputs, trace=False):
    import ml_dtypes

    from concourse.bass_utils import run_bass_kernel_spmd

    nc = _get_nc()
    pf = np.ascontiguousarray(
        np.asarray(inputs["point_features"], dtype=np.float32)
    ).astype(ml_dtypes.bfloat16)
    cfeat = np.ascontiguousarray(
        np.asarray(inputs["centroid_features"], dtype=np.float32)
    )
    wp = np.asarray(inputs["W_p"], dtype=np.float64)
    bp = np.asarray(inputs["b_p"], dtype=np.float64)
    wc = np.asarray(inputs["W_c"], dtype=np.float64)
    bc = np.asarray(inputs["b_c"], dtype=np.float64)

    # Host-fold the weight-only constants (fp64): M = W_p @ c_proj.T,
    # t = b_p @ c_proj.T, duplicated along k so S^T lands twice.
    # cfs [128, C+2] bf16: rows 0-63 = [bf16(CF) | 1 | 1], rows 64-127 =
    # [bf16(CF - bf16(CF)) | 0 | 0]; the duplicated expT contracts hi+lo
    # so CF keeps ~16 mantissa bits through the bf16 weighted matmul.
    in_maps = []
    for b in range(B):
        cproj = cfeat[b].astype(np.float64) @ wc + bc  # (NC, C)
        m = (wp @ cproj.T).astype(ml_dtypes.bfloat16)  # (C, NC)
        t = (bp @ cproj.T).astype(np.float32)  # (NC,)
        mcd = np.ascontiguousarray(np.concatenate([m, m], axis=1))
        td = np.concatenate([t, t]).reshape(P, 1)
        cf64 = cfeat[b].astype(np.float64)  # (NC, C)
        cf_hi = cf64.astype(ml_dtypes.bfloat16)
        cf_lo = (cf64 - cf_hi.astype(np.float64)).astype(ml_dtypes.bfloat16)
        cfs = np.zeros((P, C + 2), dtype=ml_dtypes.bfloat16)
        cfs[:NC, :C] = cf_hi
        cfs[NC:, :C] = cf_lo
        cfs[:NC, C:] = ml_dtypes.bfloat16(1.0)
        in_maps.append(
            {"x": pf[b], "cfs": cfs, "mcd": mcd, "td": td}
        )
    res = run_bass_kernel_spmd(nc, in_maps, core_ids=list(range(B)), trace=trace)
    out = np.stack(
        [np.asarray(res.results[b]["out"]).astype(np.float32) for b in range(B)],
        axis=0,
    )
    return out, res


def kernel(**inputs) -> np.ndarray:
    out, _ = run(inputs, trace=False)
    return out

